# revision 16
# baseline (speedup 1.0000x reference)
"""Causal self-attention kernel for 8 Trainium2 NeuronCores.

Problem: B=4, T=2048, C=1024, H=16 heads, HD=64.
  qkv = hs @ qkv_w.T + qkv_b ; per-head causal softmax attention ;
  out = attn @ o_w.T + o_b

Sharding (8 cores): core c handles batch b = c//2 and head-half g = c%2
(8 heads). Each core computes q/k/v for its heads from its batch's
hidden states, runs causal attention, and produces a partial output
projection over its 512 attention-output channels. The host sums the
two partials per batch and adds o_b.

On-device layout/dataflow (per core), v2:
  - all matmul operands in bf16 (error ~4e-3 vs the 2e-2 gate): flat
    1 cycle/row on the PE with no fp32r N<256 penalty, half-size
    LDWEIGHTS, and 2x DVE mode for the bf16 mask multiply.
  - qT, kT stored [d, t] (d on partitions); v stored [t, d] natural,
    augmented with a ones-column so the PV matmul's row 64 accumulates
    the softmax denominator for free.
  - scores computed transposed [j, q] in PSUM; two heads share the PE
    via tile_position row packing (K=64) and one 2-bank PSUM tile so a
    single exp covers the pair; no max-subtraction (scores ~N(0,1)).
  - causal mask applied multiplicatively post-exp only on the 128-wide
    triangular sub-block of diagonal j-chunks (one [128,128] bf16
    mask table serves every diagonal block).
  - PSUM (8 banks): scores 2x[128,2,512] (4), PV pair accum
    1x[65,2,512] (2) drained to an SBUF f32 accumulator every 8
    j-chunks, filler pool 2x[128,512] (2). Fillers (QKV of the next
    chunk, o-proj of the previous) are dripped into the attention loop
    as 2-3-matmul closures so scores never starve behind them.
  - q/k bias-copies run on ACT (activation Identity with per-partition
    bias), v/o-proj copies and softmax normalize on DVE, the
    reciprocal's partition-broadcast on GpSimd.
"""
import numpy as np
from contextlib import ExitStack

import concourse.bass as bass
from concourse import bacc
import concourse.tile as tile
import concourse.mybir as mybir
from concourse.bass_utils import run_bass_kernel_spmd

B, T, C = 4, 2048, 1024
H, HD = 16, 64
NCORES = 8
HPC = H // 2            # 8 heads per core
E = HPC * HD            # 512 local attn-out channels per core
P = 128
SC = 512                # q-chunk (matmul free dim)
NQC = T // SC           # 4 q-chunks
NJC = T // P            # 16 j-chunks
CC = C // P             # 8 contraction chunks
F32 = mybir.dt.float32
BF16 = mybir.dt.bfloat16
Exp = mybir.ActivationFunctionType.Exp
Ident = mybir.ActivationFunctionType.Identity
SCALE = HD ** -0.5
GRP = 8                 # j-chunks per PV psum->sbuf flush group

_cache = {}


def _build():
    nc = bacc.Bacc("TRN2", target_bir_lowering=False, debug=False)
    hsT = nc.dram_tensor("hsT", [C, T], BF16, kind="ExternalInput")
    wqkvT = nc.dram_tensor("wqkvT", [C, 3 * E], BF16, kind="ExternalInput")
    woT = nc.dram_tensor("woT", [E, C], BF16, kind="ExternalInput")
    bqkv = nc.dram_tensor("bqkv", [P, 8], F32, kind="ExternalInput")
    vbias = nc.dram_tensor("vbias", [P, E], F32, kind="ExternalInput")
    masks = nc.dram_tensor("masks", [P, P], BF16, kind="ExternalInput")
    outp = nc.dram_tensor("outp", [T, C], F32, kind="ExternalOutput")

    with tile.TileContext(nc) as tc, ExitStack() as ctx:
        const_pool = ctx.enter_context(tc.tile_pool(name="const", bufs=1))
        qk_pool = ctx.enter_context(tc.tile_pool(name="qk", bufs=1))

        bqkv_sb = const_pool.tile([P, 8], F32)
        vbias_sb = const_pool.tile([P, E], F32)
        masks_sb = const_pool.tile([P, P], BF16)
        ones_sb = const_pool.tile([P, 1], F32)
        nc.sync.dma_start(bqkv_sb[:], bqkv.ap())
        nc.vector.memset(ones_sb[:], 1.0)

        kT = qk_pool.tile([P, 4, T], BF16)            # [d%128, d//128, t]
        v_aug = qk_pool.tile([P, NJC, HPC, HD + 1], BF16)  # [t%128, jc, h, d|1]
        nc.vector.tensor_copy(
            v_aug[:, :, :, HD], ones_sb[:, 0, None, None].to_broadcast((P, NJC, HPC))
        )

        # PSUM: scores 2x2 banks, PV pair accum 1x2 banks, fillers 2x1.
        sc_pool = ctx.enter_context(tc.tile_pool(name="scp", bufs=2, space="PSUM"))
        pv_pool = ctx.enter_context(tc.tile_pool(name="pvp", bufs=1, space="PSUM"))
        fl_pool = ctx.enter_context(tc.tile_pool(name="flp", bufs=2, space="PSUM"))

        wq_pool = ctx.enter_context(tc.tile_pool(name="wq", bufs=1))
        hst_pool = ctx.enter_context(tc.tile_pool(name="hst", bufs=2))
        qt_pool = ctx.enter_context(tc.tile_pool(name="qtp", bufs=2))
        wo_pool = ctx.enter_context(tc.tile_pool(name="wo", bufs=1))
        attnp_pool = ctx.enter_context(tc.tile_pool(name="attnp", bufs=8))
        exp_pool = ctx.enter_context(tc.tile_pool(name="expp", bufs=8))
        acc_pool = ctx.enter_context(tc.tile_pool(name="accp", bufs=3))
        bc_pool = ctx.enter_context(tc.tile_pool(name="bcp", bufs=2))
        rc_pool = ctx.enter_context(tc.tile_pool(name="rcp", bufs=2))
        ost_pool = ctx.enter_context(tc.tile_pool(name="ost", bufs=2))
        qTcs = {}

        wqkvT_sb = wq_pool.tile([P, CC, 3 * E], BF16)
        wq_src = wqkvT.ap().rearrange("(co p) d -> p co d", p=P)
        hst0 = hst_pool.tile([P, CC, SC], BF16, tag="hst", name="hst")
        hs_src0 = hsT.ap()[:, 0:SC].rearrange("(co p) t -> p co t", p=P)
        # startup-priority order: hs chunk + q/k weights gate the first
        # QKV units. Dispatch across three engine queues in parallel
        # (the Sync queue serializes dispatches at ~650ns each).
        for cc in range(CC):
            nc.sync.dma_start(hst0[:, cc], hs_src0[:, cc])
            nc.scalar.dma_start(wqkvT_sb[:, cc, 0:E], wq_src[:, cc, 0:E])
            nc.gpsimd.dma_start(wqkvT_sb[:, cc, E:2 * E], wq_src[:, cc, E:2 * E])
        for cc in range(CC):
            nc.sync.dma_start(
                wqkvT_sb[:, cc, 2 * E:3 * E], wq_src[:, cc, 2 * E:3 * E]
            )
        nc.sync.dma_start(vbias_sb[:], vbias.ap())
        nc.sync.dma_start(masks_sb[:], masks.ap())
        woT_sb = wo_pool.tile([P, E // P, C], BF16)
        nc.sync.dma_start(woT_sb[:], woT.ap().rearrange("(ec p) co -> p ec co", p=P))

        def qkv_closures(t4):
            """QKV for hs chunk t4 as a list of fine-grained closures
            (<=3 matmuls each). Must be preceded by qkv_load(t4)."""
            st = {}

            def mk_qk(qk, blk):
                # one 128-channel block of q or k: 8 cc-matmuls into a
                # 1-bank psum tile + one ACT bias-copy to bf16 SBUF
                def mm(c0):
                    if c0 == 0:
                        st[(qk, blk)] = fl_pool.tile(
                            [P, SC], F32, tag="fl", name="flqk"
                        )
                    ps = st[(qk, blk)]
                    w0 = qk * E + blk * P
                    for cc in range(c0, min(c0 + 3, CC)):
                        nc.tensor.matmul(
                            ps[:], wqkvT_sb[:, cc, w0:w0 + P], st["hst"][:, cc, :],
                            start=(cc == 0), stop=(cc == CC - 1),
                        )

                def fin():
                    ps = st.pop((qk, blk))
                    dst = (qTcs[t4][:, blk, :] if qk == 0 else
                           kT[:, blk, t4 * SC:(t4 + 1) * SC])
                    nc.vector.tensor_add(
                        dst, ps[:],
                        bqkv_sb[:, qk * 4 + blk, None].to_broadcast((P, SC)),
                    )

                return [lambda: mm(0), lambda: mm(3), lambda: (mm(6), fin())]

            def mk_v(ts):
                # one 128-row (t) block of v: 8 cc-matmuls + DVE bias add
                def mm(c0):
                    if c0 == 0:
                        st[("v", ts)] = fl_pool.tile(
                            [P, E], F32, tag="fl", name="flv"
                        )
                    ps = st[("v", ts)]
                    for cc in range(c0, min(c0 + 3, CC)):
                        nc.tensor.matmul(
                            ps[:], st["hst"][:, cc, ts * P:(ts + 1) * P],
                            wqkvT_sb[:, cc, 2 * E:3 * E],
                            start=(cc == 0), stop=(cc == CC - 1),
                        )

                def fin():
                    ps = st.pop(("v", ts))
                    jc = t4 * 4 + ts
                    nc.vector.tensor_add(
                        v_aug[:, jc, :, 0:HD],
                        ps[:].rearrange("p (h d) -> p h d", d=HD),
                        vbias_sb.rearrange("p (h d) -> p h d", d=HD),
                    )

                return [lambda: mm(0), lambda: mm(3), lambda: (mm(6), fin())]

            def load():
                if t4 == 0:
                    st["hst"] = hst0
                else:
                    hst = hst_pool.tile([P, CC, SC], BF16, tag="hst", name="hst")
                    nc.sync.dma_start(
                        hst[:],
                        hsT.ap()[:, t4 * SC:(t4 + 1) * SC].rearrange(
                            "(co p) t -> p co t", p=P
                        ),
                    )
                    st["hst"] = hst
                qTcs[t4] = qt_pool.tile([P, 4, SC], BF16, tag="qTc", name="qTc")

            cls_qk = []
            for qk in (0, 1):
                for blk in range(4):
                    cls_qk.extend(mk_qk(qk, blk))
            cls_v = []
            for ts in range(4):
                cls_v.extend(mk_v(ts))
            return load, cls_qk, cls_v

        def oproj_closures(qc, attnp):
            """o-proj for q-chunk qc: per (t8, co) a 1-bank unit of
            4 matmuls + copy + dma, split into 2 closures."""
            st = {}

            def mm(t8, co, e0):
                if e0 == 0:
                    st[(t8, co)] = fl_pool.tile([P, SC], F32, tag="fl", name="flo")
                ps = st[(t8, co)]
                for ec in range(e0, e0 + 2):
                    nc.tensor.matmul(
                        ps[:], attnp[ec][:, t8 * P:(t8 + 1) * P],
                        woT_sb[:, ec, co * SC:(co + 1) * SC],
                        start=(ec == 0), stop=(ec == E // P - 1),
                    )

            def fin(t8, co):
                ps = st.pop((t8, co))
                trow = qc * SC + t8 * P
                sbo = ost_pool.tile([P, SC], F32, tag="ost")
                nc.vector.tensor_copy(sbo[:], ps[:])
                nc.sync.dma_start(
                    outp.ap()[trow:trow + P, co * SC:(co + 1) * SC], sbo[:]
                )

            cls = []
            for t8 in range(SC // P):
                for co in range(2):
                    cls.append(lambda t8=t8, co=co: mm(t8, co, 0))
                    cls.append(lambda t8=t8, co=co: (mm(t8, co, 2), fin(t8, co)))
            return cls

        def emit_attn(qc, hp, attnp, step_hook=None):
            nj = 4 * (qc + 1)
            acc = acc_pool.tile([HD + 1, 2, SC], F32, tag="acc", name="acc")
            st = {"g0": 0, "pv": None}

            def flush(glast):
                # drain the PV psum pair accum into the SBUF accumulator
                pv = st["pv"]
                st["pv"] = None
                if st["g0"] == 0:
                    nc.vector.tensor_copy(acc[:], pv[:])
                else:
                    nc.vector.tensor_add(acc[:], acc[:], pv[:])
                st["g0"] = glast

            def emit_pv(item):
                jc, n0, e = item
                if st["pv"] is None:
                    st["pv"] = pv_pool.tile(
                        [HD + 1, 2, SC], F32, tag="pv", name="pvps"
                    )
                gend = min(st["g0"] + GRP, nj)
                for s in range(2):
                    nc.tensor.matmul(
                        st["pv"][:, s, n0:SC], v_aug[:, jc, 2 * hp + s, :],
                        e[:, s, n0:SC],
                        start=(jc == st["g0"]), stop=(jc == gend - 1),
                    )
                if jc == gend - 1:
                    flush(gend)

            pends = []  # deferred PV pairs (software pipeline skew)
            for jc in range(nj):
                di = jc - 4 * qc  # >= 0 on diagonal-straddling chunks
                n0 = P * di if di >= 0 else 0
                j0 = jc * P
                sc_ps = sc_pool.tile([P, 2, SC], F32, tag="sc", name="scps")
                for s in range(2):
                    nc.tensor.matmul(
                        sc_ps[:, s, n0:SC],
                        kT[64 * s:64 * s + 64, hp, j0:j0 + P],
                        qTcs[qc][64 * s:64 * s + 64, hp, n0:SC],
                        start=True, stop=True, tile_position=(64 * s, 0),
                    )
                e = exp_pool.tile([P, 2, SC], BF16, tag="exp")
                nc.scalar.activation(
                    e[:, :, n0:SC], sc_ps[:, :, n0:SC], Exp, scale=SCALE
                )
                if di >= 0:
                    # only the 128-wide diagonal sub-block needs masking
                    nc.vector.tensor_mul(
                        e[:, :, n0:n0 + P], e[:, :, n0:n0 + P],
                        masks_sb[:, None].to_broadcast((P, 2, P)),
                    )
                if len(pends) >= 3:
                    emit_pv(pends.pop(0))
                if step_hook is not None:
                    step_hook()
                pends.append((jc, n0, e))
            for item in pends:
                emit_pv(item)

            def norm():
                # normalize by the ones-row sum and place into attnp;
                # deferred into the next head-pair's attention stream so
                # its serial DVE/GpSimd latency hides behind PE work
                srow = rc_pool.tile([1, 2, SC], F32, tag="srow")
                nc.vector.tensor_copy(srow[:], acc[HD:HD + 1, :, :])
                rc = rc_pool.tile([1, 2, SC], F32, tag="rc")
                nc.vector.reciprocal_approx_fast(rc[:], srow[:])
                bcast = bc_pool.tile([64, 2, SC], F32, tag="bc")
                nc.gpsimd.partition_broadcast(bcast[:], rc[:])
                for s in range(2):
                    nc.vector.tensor_mul(
                        attnp[hp][64 * s:64 * s + 64, :], acc[0:HD, s, :],
                        bcast[:, s, :]
                    )

            return norm

        # software-pipelined emission: QKV for chunk t4+1 and the previous
        # q-chunk's output projection are dripped into the attention
        # j-loop as <=3-matmul closures (the PE is in-order, so filler
        # work must sit between attention steps in fine grains).
        load0, cls0_qk, cls0_v = qkv_closures(0)
        load0()
        for u in cls0_qk:
            u()  # q/k of chunk 0 up front; its v-units ride as qc0 fillers
        carry = list(cls0_v)
        extras = []  # deferred normalize closures, drained at next steps
        attnps = {}
        for qc in range(NQC):
            attnps[qc] = [
                attnp_pool.tile([P, SC], BF16, tag="attnp", name="attnp")
                for _ in range(4)
            ]
            fillers = list(carry)
            carry = []
            if qc + 1 < NQC:
                load, cls_qk, cls_v = qkv_closures(qc + 1)
                load()  # issue the hsT chunk DMA as early as possible
                fillers.extend(cls_qk)
                fillers.extend(cls_v)
            if qc > 0:
                fillers.extend(oproj_closures(qc - 1, attnps[qc - 1]))
            steps_total = 4 * 4 * (qc + 1)
            nun = len(fillers)
            state = {"step": 0, "done": 0}

            def step_hook():
                state["step"] += 1
                while extras:
                    extras.pop(0)()
                while (state["done"] < nun
                       and state["step"] * nun >= (state["done"] + 1) * steps_total):
                    fillers[state["done"]]()
                    state["done"] += 1

            for hp in range(4):
                extras.append(emit_attn(qc, hp, attnps[qc], step_hook))
            for u in fillers[state["done"]:]:
                u()
        # final q-chunk o-proj: emit each unit's first-half matmuls (they
        # need only the already-normalized head-pairs) under the deferred
        # last normalize, second halves after it
        last_cls = oproj_closures(NQC - 1, attnps[NQC - 1])
        pre = last_cls[0::2]
        post = last_cls[1::2]
        pre[0]()
        pre[1]()
        while extras:
            extras.pop(0)()
        for i, u in enumerate(post):
            u()
            if i + 2 < len(pre):
                pre[i + 2]()

    nc.compile()
    return nc


def _prep_inputs(hidden_states, qkv_w, qkv_b, o_w, o_b):
    import ml_dtypes
    bf = ml_dtypes.bfloat16
    hidden_states = np.asarray(hidden_states, dtype=np.float32)
    qkv_w = np.asarray(qkv_w, dtype=np.float32)
    qkv_b = np.asarray(qkv_b, dtype=np.float32)
    o_w = np.asarray(o_w, dtype=np.float32)

    j = np.arange(P)[:, None]
    q = np.arange(P)[None, :]
    msk = (j <= q).astype(bf)

    in_maps = []
    for c in range(NCORES):
        b, g = c // 2, c % 2
        hsT = np.ascontiguousarray(hidden_states[b].T).astype(bf)
        qsel = qkv_w[E * g:E * g + E]
        ksel = qkv_w[C + E * g:C + E * g + E]
        vsel = qkv_w[2 * C + E * g:2 * C + E * g + E]
        wqkvT = np.ascontiguousarray(
            np.concatenate([qsel, ksel, vsel], 0).T
        ).astype(bf)
        woT = np.ascontiguousarray(o_w[:, E * g:E * g + E].T).astype(bf)
        bq = qkv_b[E * g:E * g + E].reshape(4, P).T
        bk = qkv_b[C + E * g:C + E * g + E].reshape(4, P).T
        bv = qkv_b[2 * C + E * g:2 * C + E * g + E]
        bqkv = np.ascontiguousarray(np.concatenate([bq, bk], 1))
        vbias = np.ascontiguousarray(np.tile(bv[None, :], (P, 1)))
        in_maps.append({
            "hsT": hsT, "wqkvT": wqkvT, "woT": woT,
            "bqkv": bqkv, "vbias": vbias, "masks": msk,
        })
    return in_maps


def _get_nc():
    if "nc" not in _cache:
        _cache["nc"] = _build()
    return _cache["nc"]


def _run(in_maps, **kwargs):
    return run_bass_kernel_spmd(
        _get_nc(), in_maps, core_ids=list(range(NCORES)), **kwargs
    )


def kernel(hidden_states, qkv_w, qkv_b, o_w, o_b, **_):
    in_maps = _prep_inputs(hidden_states, qkv_w, qkv_b, o_w, o_b)
    res = _run(in_maps)
    o_b = np.asarray(o_b, dtype=np.float32)
    out = np.empty((B, T, C), dtype=np.float32)
    for b in range(B):
        out[b] = res.results[2 * b]["outp"] + res.results[2 * b + 1]["outp"] + o_b
    return out


# revision 20
# speedup vs baseline: 1.0189x; 1.0189x over previous
"""Causal self-attention kernel for 8 Trainium2 NeuronCores.

Problem: B=4, T=2048, C=1024, H=16 heads, HD=64.
  qkv = hs @ qkv_w.T + qkv_b ; per-head causal softmax attention ;
  out = attn @ o_w.T + o_b

Sharding (8 cores): core c handles batch b = c//2 and head-half g = c%2
(8 heads). Each core computes q/k/v for its heads from its batch's
hidden states, runs causal attention, and produces a partial output
projection over its 512 attention-output channels. The host sums the
two partials per batch and adds o_b.

On-device layout/dataflow (per core), v2:
  - all matmul operands in bf16 (error ~4e-3 vs the 2e-2 gate): flat
    1 cycle/row on the PE with no fp32r N<256 penalty, half-size
    LDWEIGHTS, and 2x DVE mode for the bf16 mask multiply.
  - qT, kT stored [d, t] (d on partitions); v stored [t, d] natural,
    augmented with a ones-column so the PV matmul's row 64 accumulates
    the softmax denominator for free.
  - scores computed transposed [j, q] in PSUM; two heads share the PE
    via tile_position row packing (K=64) and one 2-bank PSUM tile so a
    single exp covers the pair; no max-subtraction (scores ~N(0,1)).
  - causal mask applied multiplicatively post-exp only on the 128-wide
    triangular sub-block of diagonal j-chunks (one [128,128] bf16
    mask table serves every diagonal block).
  - PSUM (8 banks): scores 2x[128,2,512] (4), PV pair accum
    1x[65,2,512] (2) drained to an SBUF f32 accumulator every 8
    j-chunks, filler pool 2x[128,512] (2). Fillers (QKV of the next
    chunk, o-proj of the previous) are dripped into the attention loop
    as 2-3-matmul closures so scores never starve behind them.
  - q/k bias-copies run on ACT (activation Identity with per-partition
    bias), v/o-proj copies and softmax normalize on DVE, the
    reciprocal's partition-broadcast on GpSimd.
"""
import numpy as np
from contextlib import ExitStack

import concourse.bass as bass
from concourse import bacc
import concourse.tile as tile
import concourse.mybir as mybir
from concourse.bass_utils import run_bass_kernel_spmd

B, T, C = 4, 2048, 1024
H, HD = 16, 64
NCORES = 8
HPC = H // 2            # 8 heads per core
E = HPC * HD            # 512 local attn-out channels per core
P = 128
SC = 512                # q-chunk (matmul free dim)
NQC = T // SC           # 4 q-chunks
NJC = T // P            # 16 j-chunks
CC = C // P             # 8 contraction chunks
F32 = mybir.dt.float32
BF16 = mybir.dt.bfloat16
Exp = mybir.ActivationFunctionType.Exp
Ident = mybir.ActivationFunctionType.Identity
SCALE = HD ** -0.5
GRP = 8                 # j-chunks per PV psum->sbuf flush group

_cache = {}


def _build():
    nc = bacc.Bacc("TRN2", target_bir_lowering=False, debug=False)
    hsT = nc.dram_tensor("hsT", [C, T], BF16, kind="ExternalInput")
    wqkvT = nc.dram_tensor("wqkvT", [C, 3 * E], BF16, kind="ExternalInput")
    woT = nc.dram_tensor("woT", [E, C], BF16, kind="ExternalInput")
    bqkv = nc.dram_tensor("bqkv", [P, 8], F32, kind="ExternalInput")
    vbias = nc.dram_tensor("vbias", [P, E], F32, kind="ExternalInput")
    masks = nc.dram_tensor("masks", [P, P], BF16, kind="ExternalInput")
    outp = nc.dram_tensor("outp", [T, C], F32, kind="ExternalOutput")

    with tile.TileContext(nc) as tc, ExitStack() as ctx:
        const_pool = ctx.enter_context(tc.tile_pool(name="const", bufs=1))
        qk_pool = ctx.enter_context(tc.tile_pool(name="qk", bufs=1))

        bqkv_sb = const_pool.tile([P, 8], F32)
        vbias_sb = const_pool.tile([P, E], F32)
        masks_sb = const_pool.tile([P, P], BF16)
        ones_sb = const_pool.tile([P, 1], F32)
        nc.sync.dma_start(bqkv_sb[:], bqkv.ap())
        nc.vector.memset(ones_sb[:], 1.0)

        kT = qk_pool.tile([P, 4, T], BF16)            # [d%128, d//128, t]
        v_aug = qk_pool.tile([P, NJC, HPC, HD + 1], BF16)  # [t%128, jc, h, d|1]
        nc.vector.tensor_copy(
            v_aug[:, :, :, HD], ones_sb[:, 0, None, None].to_broadcast((P, NJC, HPC))
        )

        # PSUM: scores 2x2 banks, PV pair accum 1x2 banks, fillers 2x1.
        sc_pool = ctx.enter_context(tc.tile_pool(name="scp", bufs=2, space="PSUM"))
        pv_pool = ctx.enter_context(tc.tile_pool(name="pvp", bufs=1, space="PSUM"))
        fl_pool = ctx.enter_context(tc.tile_pool(name="flp", bufs=2, space="PSUM"))

        wq_pool = ctx.enter_context(tc.tile_pool(name="wq", bufs=1))
        hst_pool = ctx.enter_context(tc.tile_pool(name="hst", bufs=2))
        qt_pool = ctx.enter_context(tc.tile_pool(name="qtp", bufs=2))
        wo_pool = ctx.enter_context(tc.tile_pool(name="wo", bufs=1))
        attnp_pool = ctx.enter_context(tc.tile_pool(name="attnp", bufs=8))
        exp_pool = ctx.enter_context(tc.tile_pool(name="expp", bufs=8))
        acc_pool = ctx.enter_context(tc.tile_pool(name="accp", bufs=3))
        bc_pool = ctx.enter_context(tc.tile_pool(name="bcp", bufs=2))
        rc_pool = ctx.enter_context(tc.tile_pool(name="rcp", bufs=2))
        ost_pool = ctx.enter_context(tc.tile_pool(name="ost", bufs=4))
        qTcs = {}

        wqkvT_sb = wq_pool.tile([P, CC, 3 * E], BF16)
        wq_src = wqkvT.ap().rearrange("(co p) d -> p co d", p=P)
        hst0 = hst_pool.tile([P, CC, SC], BF16, tag="hst", name="hst")
        hs_src0 = hsT.ap()[:, 0:SC].rearrange("(co p) t -> p co t", p=P)
        # startup-priority order: hs chunk + q/k weights gate the first
        # QKV units. Dispatch across three engine queues in parallel
        # (the Sync queue serializes dispatches at ~650ns each).
        for cc in range(CC):
            nc.sync.dma_start(hst0[:, cc], hs_src0[:, cc])
            nc.scalar.dma_start(wqkvT_sb[:, cc, 0:E], wq_src[:, cc, 0:E])
        for cc in range(CC):
            nc.sync.dma_start(wqkvT_sb[:, cc, E:2 * E], wq_src[:, cc, E:2 * E])
        for cc in range(CC):
            nc.sync.dma_start(
                wqkvT_sb[:, cc, 2 * E:3 * E], wq_src[:, cc, 2 * E:3 * E]
            )
        nc.sync.dma_start(vbias_sb[:], vbias.ap())
        nc.sync.dma_start(masks_sb[:], masks.ap())
        woT_sb = wo_pool.tile([P, E // P, C], BF16)
        nc.sync.dma_start(woT_sb[:], woT.ap().rearrange("(ec p) co -> p ec co", p=P))

        def qkv_closures(t4):
            """QKV for hs chunk t4 as a list of fine-grained closures
            (<=3 matmuls each). Must be preceded by qkv_load(t4)."""
            st = {}

            def mk_qk(qk, blk):
                # one 128-channel block of q or k: 8 cc-matmuls into a
                # 1-bank psum tile + one ACT bias-copy to bf16 SBUF
                def mm(c0):
                    if c0 == 0:
                        st[(qk, blk)] = fl_pool.tile(
                            [P, SC], F32, tag="fl", name="flqk"
                        )
                    ps = st[(qk, blk)]
                    w0 = qk * E + blk * P
                    for cc in range(c0, min(c0 + 3, CC)):
                        nc.tensor.matmul(
                            ps[:], wqkvT_sb[:, cc, w0:w0 + P], st["hst"][:, cc, :],
                            start=(cc == 0), stop=(cc == CC - 1),
                        )

                def fin():
                    ps = st.pop((qk, blk))
                    dst = (qTcs[t4][:, blk, :] if qk == 0 else
                           kT[:, blk, t4 * SC:(t4 + 1) * SC])
                    nc.vector.tensor_add(
                        dst, ps[:],
                        bqkv_sb[:, qk * 4 + blk, None].to_broadcast((P, SC)),
                    )

                return [lambda: mm(0), lambda: mm(3), lambda: (mm(6), fin())]

            def mk_v(ts):
                # one 128-row (t) block of v: 8 cc-matmuls + DVE bias add
                def mm(c0):
                    if c0 == 0:
                        st[("v", ts)] = fl_pool.tile(
                            [P, E], F32, tag="fl", name="flv"
                        )
                    ps = st[("v", ts)]
                    for cc in range(c0, min(c0 + 3, CC)):
                        nc.tensor.matmul(
                            ps[:], st["hst"][:, cc, ts * P:(ts + 1) * P],
                            wqkvT_sb[:, cc, 2 * E:3 * E],
                            start=(cc == 0), stop=(cc == CC - 1),
                        )

                def fin():
                    ps = st.pop(("v", ts))
                    jc = t4 * 4 + ts
                    nc.vector.tensor_add(
                        v_aug[:, jc, :, 0:HD],
                        ps[:].rearrange("p (h d) -> p h d", d=HD),
                        vbias_sb.rearrange("p (h d) -> p h d", d=HD),
                    )

                return [lambda: mm(0), lambda: mm(3), lambda: (mm(6), fin())]

            def load():
                if t4 == 0:
                    st["hst"] = hst0
                else:
                    hst = hst_pool.tile([P, CC, SC], BF16, tag="hst", name="hst")
                    nc.sync.dma_start(
                        hst[:],
                        hsT.ap()[:, t4 * SC:(t4 + 1) * SC].rearrange(
                            "(co p) t -> p co t", p=P
                        ),
                    )
                    st["hst"] = hst
                qTcs[t4] = qt_pool.tile([P, 4, SC], BF16, tag="qTc", name="qTc")

            cls_qk = []
            for qk in (0, 1):
                for blk in range(4):
                    cls_qk.extend(mk_qk(qk, blk))
            cls_v = []
            for ts in range(4):
                cls_v.extend(mk_v(ts))
            return load, cls_qk, cls_v

        def oproj_closures(qc, attnp):
            """o-proj for q-chunk qc: per (t8, co) a 1-bank unit of
            4 matmuls + copy + dma, split into 2 closures."""
            st = {}

            def mm(t8, co, e0):
                if e0 == 0:
                    st[(t8, co)] = fl_pool.tile([P, SC], F32, tag="fl", name="flo")
                ps = st[(t8, co)]
                for ec in range(e0, e0 + 2):
                    nc.tensor.matmul(
                        ps[:], attnp[ec][:, t8 * P:(t8 + 1) * P],
                        woT_sb[:, ec, co * SC:(co + 1) * SC],
                        start=(ec == 0), stop=(ec == E // P - 1),
                    )

            def fin(t8, co):
                ps = st.pop((t8, co))
                trow = qc * SC + t8 * P
                sbo = ost_pool.tile([P, SC], F32, tag="ost")
                nc.vector.tensor_copy(sbo[:], ps[:])
                nc.sync.dma_start(
                    outp.ap()[trow:trow + P, co * SC:(co + 1) * SC], sbo[:]
                )

            cls = []
            for t8 in range(SC // P):
                for co in range(2):
                    cls.append(lambda t8=t8, co=co: mm(t8, co, 0))
                    cls.append(lambda t8=t8, co=co: (mm(t8, co, 2), fin(t8, co)))
            return cls

        def emit_attn(qc, hp, attnp, step_hook=None):
            nj = 4 * (qc + 1)
            acc = acc_pool.tile([HD + 1, 2, SC], F32, tag="acc", name="acc")
            st = {"g0": 0, "pv": None}

            def flush(glast):
                # drain the PV psum pair accum into the SBUF accumulator
                pv = st["pv"]
                st["pv"] = None
                if st["g0"] == 0:
                    nc.vector.tensor_copy(acc[:], pv[:])
                else:
                    nc.vector.tensor_add(acc[:], acc[:], pv[:])
                st["g0"] = glast

            def emit_pv(item):
                jc, n0, e = item
                if st["pv"] is None:
                    st["pv"] = pv_pool.tile(
                        [HD + 1, 2, SC], F32, tag="pv", name="pvps"
                    )
                gend = min(st["g0"] + GRP, nj)
                for s in range(2):
                    nc.tensor.matmul(
                        st["pv"][:, s, n0:SC], v_aug[:, jc, 2 * hp + s, :],
                        e[:, s, n0:SC],
                        start=(jc == st["g0"]), stop=(jc == gend - 1),
                    )
                if jc == gend - 1:
                    flush(gend)

            pends = []  # deferred PV pairs (software pipeline skew)
            for jc in range(nj):
                di = jc - 4 * qc  # >= 0 on diagonal-straddling chunks
                n0 = P * di if di >= 0 else 0
                j0 = jc * P
                sc_ps = sc_pool.tile([P, 2, SC], F32, tag="sc", name="scps")
                for s in range(2):
                    nc.tensor.matmul(
                        sc_ps[:, s, n0:SC],
                        kT[64 * s:64 * s + 64, hp, j0:j0 + P],
                        qTcs[qc][64 * s:64 * s + 64, hp, n0:SC],
                        start=True, stop=True, tile_position=(64 * s, 0),
                    )
                e = exp_pool.tile([P, 2, SC], BF16, tag="exp")
                nc.scalar.activation(
                    e[:, :, n0:SC], sc_ps[:, :, n0:SC], Exp, scale=SCALE
                )
                if di >= 0:
                    # only the 128-wide diagonal sub-block needs masking
                    nc.vector.tensor_mul(
                        e[:, :, n0:n0 + P], e[:, :, n0:n0 + P],
                        masks_sb[:, None].to_broadcast((P, 2, P)),
                    )
                if len(pends) >= 3:
                    emit_pv(pends.pop(0))
                if step_hook is not None:
                    step_hook()
                pends.append((jc, n0, e))
            for item in pends:
                emit_pv(item)

            def norm():
                # normalize by the ones-row sum and place into attnp;
                # deferred into the next head-pair's attention stream so
                # its serial DVE/GpSimd latency hides behind PE work
                srow = rc_pool.tile([1, 2, SC], F32, tag="srow")
                nc.vector.tensor_copy(srow[:], acc[HD:HD + 1, :, :])
                rc = rc_pool.tile([1, 2, SC], F32, tag="rc")
                nc.vector.reciprocal_approx_fast(rc[:], srow[:])
                bcast = bc_pool.tile([64, 2, SC], F32, tag="bc")
                nc.gpsimd.partition_broadcast(bcast[:], rc[:])
                for s in range(2):
                    nc.vector.tensor_mul(
                        attnp[hp][64 * s:64 * s + 64, :], acc[0:HD, s, :],
                        bcast[:, s, :]
                    )

            return norm

        # software-pipelined emission: QKV for chunk t4+1 and the previous
        # q-chunk's output projection are dripped into the attention
        # j-loop as <=3-matmul closures (the PE is in-order, so filler
        # work must sit between attention steps in fine grains).
        load0, cls0_qk, cls0_v = qkv_closures(0)
        load0()
        for u in cls0_qk:
            u()  # q/k of chunk 0 up front; its v-units ride as qc0 fillers
        carry = list(cls0_v)
        extras = []  # deferred normalize closures, drained at next steps
        attnps = {}
        for qc in range(NQC):
            attnps[qc] = [
                attnp_pool.tile([P, SC], BF16, tag="attnp", name="attnp")
                for _ in range(4)
            ]
            fillers = list(carry)
            carry = []
            if qc + 1 < NQC:
                load, cls_qk, cls_v = qkv_closures(qc + 1)
                load()  # issue the hsT chunk DMA as early as possible
                fillers.extend(cls_qk)
                fillers.extend(cls_v)
            if qc > 0:
                fillers.extend(oproj_closures(qc - 1, attnps[qc - 1]))
            steps_total = 4 * 4 * (qc + 1)
            nun = len(fillers)
            state = {"step": 0, "done": 0}

            def step_hook():
                state["step"] += 1
                while extras:
                    extras.pop(0)()
                while (state["done"] < nun
                       and state["step"] * nun >= (state["done"] + 1) * steps_total):
                    fillers[state["done"]]()
                    state["done"] += 1

            for hp in range(4):
                extras.append(emit_attn(qc, hp, attnps[qc], step_hook))
            for u in fillers[state["done"]:]:
                u()
        # final q-chunk o-proj: each unit becomes TWO independent psum
        # accumulation groups (ec0-1 / ec2-3, merged by the fin ADD) so
        # the scheduler can run the first group's matmuls under the
        # deferred last normalize (a single spanning group would be kept
        # atomic and pushed wholly after it)
        attnp3 = attnps[NQC - 1]
        fst = {}

        def f_mm(t8, co, g):
            ps = fl_pool.tile([P, SC], F32, tag="fl", name="fsc")
            fst[(t8, co, g)] = ps
            for ec in (2 * g, 2 * g + 1):
                nc.tensor.matmul(
                    ps[:], attnp3[ec][:, t8 * P:(t8 + 1) * P],
                    woT_sb[:, ec, co * SC:(co + 1) * SC],
                    start=(ec == 2 * g), stop=(ec == 2 * g + 1),
                )
            if g == 0:
                sbo = ost_pool.tile([P, SC], F32, tag="ost")
                fst[(t8, co, "sb")] = sbo
                nc.vector.tensor_copy(sbo[:], ps[:])

        def f_fin(t8, co):
            psB = fst.pop((t8, co, 1))
            sbo = fst.pop((t8, co, "sb"))
            trow = (NQC - 1) * SC + t8 * P
            nc.vector.tensor_add(sbo[:], sbo[:], psB[:])
            nc.sync.dma_start(
                outp.ap()[trow:trow + P, co * SC:(co + 1) * SC], sbo[:]
            )

        units = [(t8, co) for t8 in range(SC // P) for co in range(2)]
        f_mm(*units[0], 0)
        f_mm(*units[1], 0)
        while extras:
            extras.pop(0)()
        for i, (t8, co) in enumerate(units):
            f_mm(t8, co, 1)
            f_fin(t8, co)
            if i + 2 < len(units):
                f_mm(*units[i + 2], 0)

    nc.compile()
    return nc


def _prep_inputs(hidden_states, qkv_w, qkv_b, o_w, o_b):
    import ml_dtypes
    bf = ml_dtypes.bfloat16
    hidden_states = np.asarray(hidden_states, dtype=np.float32)
    qkv_w = np.asarray(qkv_w, dtype=np.float32)
    qkv_b = np.asarray(qkv_b, dtype=np.float32)
    o_w = np.asarray(o_w, dtype=np.float32)

    j = np.arange(P)[:, None]
    q = np.arange(P)[None, :]
    msk = (j <= q).astype(bf)

    in_maps = []
    for c in range(NCORES):
        b, g = c // 2, c % 2
        hsT = np.ascontiguousarray(hidden_states[b].T).astype(bf)
        qsel = qkv_w[E * g:E * g + E]
        ksel = qkv_w[C + E * g:C + E * g + E]
        vsel = qkv_w[2 * C + E * g:2 * C + E * g + E]
        wqkvT = np.ascontiguousarray(
            np.concatenate([qsel, ksel, vsel], 0).T
        ).astype(bf)
        woT = np.ascontiguousarray(o_w[:, E * g:E * g + E].T).astype(bf)
        bq = qkv_b[E * g:E * g + E].reshape(4, P).T
        bk = qkv_b[C + E * g:C + E * g + E].reshape(4, P).T
        bv = qkv_b[2 * C + E * g:2 * C + E * g + E]
        bqkv = np.ascontiguousarray(np.concatenate([bq, bk], 1))
        vbias = np.ascontiguousarray(np.tile(bv[None, :], (P, 1)))
        in_maps.append({
            "hsT": hsT, "wqkvT": wqkvT, "woT": woT,
            "bqkv": bqkv, "vbias": vbias, "masks": msk,
        })
    return in_maps


def _get_nc():
    if "nc" not in _cache:
        _cache["nc"] = _build()
    return _cache["nc"]


def _run(in_maps, **kwargs):
    return run_bass_kernel_spmd(
        _get_nc(), in_maps, core_ids=list(range(NCORES)), **kwargs
    )


def kernel(hidden_states, qkv_w, qkv_b, o_w, o_b, **_):
    in_maps = _prep_inputs(hidden_states, qkv_w, qkv_b, o_w, o_b)
    res = _run(in_maps)
    o_b = np.asarray(o_b, dtype=np.float32)
    out = np.empty((B, T, C), dtype=np.float32)
    for b in range(B):
        out[b] = res.results[2 * b]["outp"] + res.results[2 * b + 1]["outp"] + o_b
    return out


# revision 24
# speedup vs baseline: 1.0211x; 1.0022x over previous
"""Causal self-attention kernel for 8 Trainium2 NeuronCores.

Problem: B=4, T=2048, C=1024, H=16 heads, HD=64.
  qkv = hs @ qkv_w.T + qkv_b ; per-head causal softmax attention ;
  out = attn @ o_w.T + o_b

Sharding (8 cores): core c handles batch b = c//2 and head-half g = c%2
(8 heads). Each core computes q/k/v for its heads from its batch's
hidden states, runs causal attention, and produces a partial output
projection over its 512 attention-output channels. The host sums the
two partials per batch and adds o_b.

On-device layout/dataflow (per core), v2:
  - all matmul operands in bf16 (error ~4e-3 vs the 2e-2 gate): flat
    1 cycle/row on the PE with no fp32r N<256 penalty, half-size
    LDWEIGHTS, and 2x DVE mode for the bf16 mask multiply.
  - qT, kT stored [d, t] (d on partitions); v stored [t, d] natural,
    augmented with a ones-column so the PV matmul's row 64 accumulates
    the softmax denominator for free.
  - scores computed transposed [j, q] in PSUM; two heads share the PE
    via tile_position row packing (K=64) and one 2-bank PSUM tile so a
    single exp covers the pair; no max-subtraction (scores ~N(0,1)).
  - causal mask applied multiplicatively post-exp only on the 128-wide
    triangular sub-block of diagonal j-chunks (one [128,128] bf16
    mask table serves every diagonal block).
  - PSUM (8 banks): scores 2x[128,2,512] (4), PV pair accum
    1x[65,2,512] (2) drained to an SBUF f32 accumulator every 8
    j-chunks, filler pool 2x[128,512] (2). Fillers (QKV of the next
    chunk, o-proj of the previous) are dripped into the attention loop
    as 2-3-matmul closures so scores never starve behind them.
  - bias-copies, o-proj copies and softmax normalize on DVE, the
    reciprocal's partition-broadcast on GpSimd; each head-pair's
    normalize is deferred into the next head-pair's attention stream.
  - final o-proj units are split into two independent psum groups
    (ec0-1 / ec2-3, merged by an SBUF add) so their first halves can
    overlap the last normalize chain.
"""
import numpy as np
from contextlib import ExitStack

import concourse.bass as bass
from concourse import bacc
import concourse.tile as tile
import concourse.mybir as mybir
from concourse.bass_utils import run_bass_kernel_spmd

B, T, C = 4, 2048, 1024
H, HD = 16, 64
NCORES = 8
HPC = H // 2            # 8 heads per core
E = HPC * HD            # 512 local attn-out channels per core
P = 128
SC = 512                # q-chunk (matmul free dim)
NQC = T // SC           # 4 q-chunks
NJC = T // P            # 16 j-chunks
CC = C // P             # 8 contraction chunks
F32 = mybir.dt.float32
BF16 = mybir.dt.bfloat16
Exp = mybir.ActivationFunctionType.Exp
Ident = mybir.ActivationFunctionType.Identity
SCALE = HD ** -0.5
GRP = 8                 # j-chunks per PV psum->sbuf flush group

_cache = {}


def _build():
    nc = bacc.Bacc("TRN2", target_bir_lowering=False, debug=False)
    hsT = nc.dram_tensor("hsT", [C, T], BF16, kind="ExternalInput")
    wqkvT = nc.dram_tensor("wqkvT", [C, 3 * E], BF16, kind="ExternalInput")
    woT = nc.dram_tensor("woT", [E, C], BF16, kind="ExternalInput")
    bqkv = nc.dram_tensor("bqkv", [P, 8], F32, kind="ExternalInput")
    vbias = nc.dram_tensor("vbias", [P, E], F32, kind="ExternalInput")
    masks = nc.dram_tensor("masks", [P, P], BF16, kind="ExternalInput")
    outp = nc.dram_tensor("outp", [T, C], F32, kind="ExternalOutput")

    with tile.TileContext(nc) as tc, ExitStack() as ctx:
        const_pool = ctx.enter_context(tc.tile_pool(name="const", bufs=1))
        qk_pool = ctx.enter_context(tc.tile_pool(name="qk", bufs=1))

        bqkv_sb = const_pool.tile([P, 8], F32)
        vbias_sb = const_pool.tile([P, E], F32)
        masks_sb = const_pool.tile([P, P], BF16)
        ones_sb = const_pool.tile([P, 1], F32)
        ones64_sb = const_pool.tile([1, 64], BF16)
        nc.sync.dma_start(bqkv_sb[:], bqkv.ap())
        nc.vector.memset(ones_sb[:], 1.0)
        nc.vector.memset(ones64_sb[:], 1.0)

        kT = qk_pool.tile([P, 4, T], BF16)            # [d%128, d//128, t]
        v_aug = qk_pool.tile([P, NJC, HPC, HD + 1], BF16)  # [t%128, jc, h, d|1]
        nc.vector.tensor_copy(
            v_aug[:, :, :, HD], ones_sb[:, 0, None, None].to_broadcast((P, NJC, HPC))
        )

        # PSUM: scores 2x2 banks, PV pair accum 1x2 banks, fillers 2x1.
        sc_pool = ctx.enter_context(tc.tile_pool(name="scp", bufs=2, space="PSUM"))
        pv_pool = ctx.enter_context(tc.tile_pool(name="pvp", bufs=1, space="PSUM"))
        fl_pool = ctx.enter_context(tc.tile_pool(name="flp", bufs=2, space="PSUM"))

        wq_pool = ctx.enter_context(tc.tile_pool(name="wq", bufs=1))
        hst_pool = ctx.enter_context(tc.tile_pool(name="hst", bufs=2))
        qt_pool = ctx.enter_context(tc.tile_pool(name="qtp", bufs=2))
        wo_pool = ctx.enter_context(tc.tile_pool(name="wo", bufs=1))
        attnp_pool = ctx.enter_context(tc.tile_pool(name="attnp", bufs=8))
        exp_pool = ctx.enter_context(tc.tile_pool(name="expp", bufs=8))
        acc_pool = ctx.enter_context(tc.tile_pool(name="accp", bufs=3))
        bc_pool = ctx.enter_context(tc.tile_pool(name="bcp", bufs=2))
        rc_pool = ctx.enter_context(tc.tile_pool(name="rcp", bufs=2))
        ost_pool = ctx.enter_context(tc.tile_pool(name="ost", bufs=4))
        qTcs = {}

        wqkvT_sb = wq_pool.tile([P, CC, 3 * E], BF16)
        wq_src = wqkvT.ap().rearrange("(co p) d -> p co d", p=P)
        hst0 = hst_pool.tile([P, CC, SC], BF16, tag="hst", name="hst")
        hs_src0 = hsT.ap()[:, 0:SC].rearrange("(co p) t -> p co t", p=P)
        # startup-priority order: hs chunk + q/k weights gate the first
        # QKV units. Dispatch across three engine queues in parallel
        # (the Sync queue serializes dispatches at ~650ns each).
        for cc in range(CC):
            nc.sync.dma_start(hst0[:, cc], hs_src0[:, cc])
            nc.scalar.dma_start(wqkvT_sb[:, cc, 0:E], wq_src[:, cc, 0:E])
        for cc in range(CC):
            nc.sync.dma_start(wqkvT_sb[:, cc, E:2 * E], wq_src[:, cc, E:2 * E])
        for cc in range(CC):
            nc.sync.dma_start(
                wqkvT_sb[:, cc, 2 * E:3 * E], wq_src[:, cc, 2 * E:3 * E]
            )
        nc.sync.dma_start(vbias_sb[:], vbias.ap())
        nc.sync.dma_start(masks_sb[:], masks.ap())
        woT_sb = wo_pool.tile([P, E // P, C], BF16)
        nc.sync.dma_start(woT_sb[:], woT.ap().rearrange("(ec p) co -> p ec co", p=P))

        def qkv_closures(t4):
            """QKV for hs chunk t4 as a list of fine-grained closures
            (<=3 matmuls each). Must be preceded by qkv_load(t4)."""
            st = {}

            def mk_qk(qk, blk):
                # one 128-channel block of q or k: 8 cc-matmuls into a
                # 1-bank psum tile + one ACT bias-copy to bf16 SBUF
                def mm(c0):
                    if c0 == 0:
                        st[(qk, blk)] = fl_pool.tile(
                            [P, SC], F32, tag="fl", name="flqk"
                        )
                    ps = st[(qk, blk)]
                    w0 = qk * E + blk * P
                    for cc in range(c0, min(c0 + 3, CC)):
                        nc.tensor.matmul(
                            ps[:], wqkvT_sb[:, cc, w0:w0 + P], st["hst"][:, cc, :],
                            start=(cc == 0), stop=(cc == CC - 1),
                        )

                def fin():
                    ps = st.pop((qk, blk))
                    dst = (qTcs[t4][:, blk, :] if qk == 0 else
                           kT[:, blk, t4 * SC:(t4 + 1) * SC])
                    nc.vector.tensor_add(
                        dst, ps[:],
                        bqkv_sb[:, qk * 4 + blk, None].to_broadcast((P, SC)),
                    )

                return [lambda: mm(0), lambda: mm(3), lambda: (mm(6), fin())]

            def mk_v(ts):
                # one 128-row (t) block of v: 8 cc-matmuls + DVE bias add
                def mm(c0):
                    if c0 == 0:
                        st[("v", ts)] = fl_pool.tile(
                            [P, E], F32, tag="fl", name="flv"
                        )
                    ps = st[("v", ts)]
                    for cc in range(c0, min(c0 + 3, CC)):
                        nc.tensor.matmul(
                            ps[:], st["hst"][:, cc, ts * P:(ts + 1) * P],
                            wqkvT_sb[:, cc, 2 * E:3 * E],
                            start=(cc == 0), stop=(cc == CC - 1),
                        )

                def fin():
                    ps = st.pop(("v", ts))
                    jc = t4 * 4 + ts
                    nc.vector.tensor_add(
                        v_aug[:, jc, :, 0:HD],
                        ps[:].rearrange("p (h d) -> p h d", d=HD),
                        vbias_sb.rearrange("p (h d) -> p h d", d=HD),
                    )

                return [lambda: mm(0), lambda: mm(3), lambda: (mm(6), fin())]

            def load():
                if t4 == 0:
                    st["hst"] = hst0
                else:
                    hst = hst_pool.tile([P, CC, SC], BF16, tag="hst", name="hst")
                    nc.sync.dma_start(
                        hst[:],
                        hsT.ap()[:, t4 * SC:(t4 + 1) * SC].rearrange(
                            "(co p) t -> p co t", p=P
                        ),
                    )
                    st["hst"] = hst
                qTcs[t4] = qt_pool.tile([P, 4, SC], BF16, tag="qTc", name="qTc")

            cls_qk = []
            for qk in (0, 1):
                for blk in range(4):
                    cls_qk.extend(mk_qk(qk, blk))
            cls_v = []
            for ts in range(4):
                cls_v.extend(mk_v(ts))
            return load, cls_qk, cls_v

        def oproj_closures(qc, attnp):
            """o-proj for q-chunk qc: per (t8, co) a 1-bank unit of
            4 matmuls + copy + dma, split into 2 closures."""
            st = {}

            def mm(t8, co, e0):
                if e0 == 0:
                    st[(t8, co)] = fl_pool.tile([P, SC], F32, tag="fl", name="flo")
                ps = st[(t8, co)]
                for ec in range(e0, e0 + 2):
                    nc.tensor.matmul(
                        ps[:], attnp[ec][:, t8 * P:(t8 + 1) * P],
                        woT_sb[:, ec, co * SC:(co + 1) * SC],
                        start=(ec == 0), stop=(ec == E // P - 1),
                    )

            def fin(t8, co):
                ps = st.pop((t8, co))
                trow = qc * SC + t8 * P
                sbo = ost_pool.tile([P, SC], F32, tag="ost")
                nc.vector.tensor_copy(sbo[:], ps[:])
                nc.sync.dma_start(
                    outp.ap()[trow:trow + P, co * SC:(co + 1) * SC], sbo[:]
                )

            cls = []
            for t8 in range(SC // P):
                for co in range(2):
                    cls.append(lambda t8=t8, co=co: mm(t8, co, 0))
                    cls.append(lambda t8=t8, co=co: (mm(t8, co, 2), fin(t8, co)))
            return cls

        def emit_attn(qc, hp, attnp, step_hook=None, last=False):
            nj = 4 * (qc + 1)
            acc = acc_pool.tile([HD + 1, 2, SC], F32, tag="acc", name="acc")
            st = {"g0": 0, "pv": None}

            def flush(glast):
                # drain the PV psum pair accum into the SBUF accumulator
                pv = st["pv"]
                st["pv"] = None
                if st["g0"] == 0:
                    nc.vector.tensor_copy(acc[:], pv[:])
                else:
                    nc.vector.tensor_add(acc[:], acc[:], pv[:])
                st["g0"] = glast

            def emit_pv(item):
                jc, n0, e = item
                if st["pv"] is None:
                    st["pv"] = pv_pool.tile(
                        [HD + 1, 2, SC], F32, tag="pv", name="pvps"
                    )
                gend = min(st["g0"] + GRP, nj)
                for s in range(2):
                    nc.tensor.matmul(
                        st["pv"][:, s, n0:SC], v_aug[:, jc, 2 * hp + s, :],
                        e[:, s, n0:SC],
                        start=(jc == st["g0"]), stop=(jc == gend - 1),
                    )
                if jc == gend - 1:
                    flush(gend)

            pends = []  # deferred PV pairs (software pipeline skew)
            for jc in range(nj):
                di = jc - 4 * qc  # >= 0 on diagonal-straddling chunks
                n0 = P * di if di >= 0 else 0
                j0 = jc * P
                sc_ps = sc_pool.tile([P, 2, SC], F32, tag="sc", name="scps")
                for s in range(2):
                    nc.tensor.matmul(
                        sc_ps[:, s, n0:SC],
                        kT[64 * s:64 * s + 64, hp, j0:j0 + P],
                        qTcs[qc][64 * s:64 * s + 64, hp, n0:SC],
                        start=True, stop=True, tile_position=(64 * s, 0),
                    )
                e = exp_pool.tile([P, 2, SC], BF16, tag="exp")
                nc.scalar.activation(
                    e[:, :, n0:SC], sc_ps[:, :, n0:SC], Exp, scale=SCALE
                )
                if di >= 0:
                    # only the 128-wide diagonal sub-block needs masking
                    nc.vector.tensor_mul(
                        e[:, :, n0:n0 + P], e[:, :, n0:n0 + P],
                        masks_sb[:, None].to_broadcast((P, 2, P)),
                    )
                if len(pends) >= 3:
                    emit_pv(pends.pop(0))
                if step_hook is not None:
                    step_hook()
                pends.append((jc, n0, e))
            for item in pends:
                emit_pv(item)

            def norm():
                # normalize by the ones-row sum and place into attnp;
                # deferred into the next head-pair's attention stream so
                # its serial DVE/GpSimd latency hides behind PE work
                if last:
                    # end of kernel: the GpSimd broadcast hop costs
                    # ~2.5us of serial latency; broadcast the (bf16)
                    # denominator row via a PE ones-matmul instead and
                    # take the reciprocal of the broadcast psum
                    srow_bf = rc_pool.tile([1, 2, SC], BF16, tag="srow")
                    nc.vector.tensor_copy(srow_bf[:], acc[HD:HD + 1, :, :])
                    den_ps = sc_pool.tile([P, 2, SC], F32, tag="sc",
                                          name="bcps")
                    for s in range(2):
                        nc.tensor.matmul(
                            den_ps[0:64, s, :], ones64_sb[:],
                            srow_bf[:, s, :], start=True, stop=True,
                        )
                    bcast = bc_pool.tile([64, 2, SC], F32, tag="bc")
                    nc.vector.reciprocal_approx_fast(
                        bcast[:], den_ps[0:64, :, :]
                    )
                else:
                    srow = rc_pool.tile([1, 2, SC], F32, tag="srow")
                    nc.vector.tensor_copy(srow[:], acc[HD:HD + 1, :, :])
                    rc = rc_pool.tile([1, 2, SC], F32, tag="rc")
                    nc.vector.reciprocal_approx_fast(rc[:], srow[:])
                    bcast = bc_pool.tile([64, 2, SC], F32, tag="bc")
                    nc.gpsimd.partition_broadcast(bcast[:], rc[:])
                for s in range(2):
                    nc.vector.tensor_mul(
                        attnp[hp][64 * s:64 * s + 64, :], acc[0:HD, s, :],
                        bcast[0:64, s, :]
                    )

            return norm

        # software-pipelined emission: QKV for chunk t4+1 and the previous
        # q-chunk's output projection are dripped into the attention
        # j-loop as <=3-matmul closures (the PE is in-order, so filler
        # work must sit between attention steps in fine grains).
        load0, cls0_qk, cls0_v = qkv_closures(0)
        load0()
        for u in cls0_qk:
            u()  # q/k of chunk 0 up front; its v-units ride as qc0 fillers
        carry = list(cls0_v)
        extras = []  # deferred normalize closures, drained at next steps
        attnps = {}
        for qc in range(NQC):
            attnps[qc] = [
                attnp_pool.tile([P, SC], BF16, tag="attnp", name="attnp")
                for _ in range(4)
            ]
            fillers = list(carry)
            carry = []
            if qc + 1 < NQC:
                load, cls_qk, cls_v = qkv_closures(qc + 1)
                load()  # issue the hsT chunk DMA as early as possible
                fillers.extend(cls_qk)
                fillers.extend(cls_v)
            if qc > 0:
                fillers.extend(oproj_closures(qc - 1, attnps[qc - 1]))
            steps_total = 4 * 4 * (qc + 1)
            nun = len(fillers)
            state = {"step": 0, "done": 0}

            def step_hook():
                state["step"] += 1
                while extras:
                    extras.pop(0)()
                while (state["done"] < nun
                       and state["step"] * nun >= (state["done"] + 1) * steps_total):
                    fillers[state["done"]]()
                    state["done"] += 1

            for hp in range(4):
                extras.append(emit_attn(
                    qc, hp, attnps[qc], step_hook,
                    last=(qc == NQC - 1 and hp == 3),
                ))
            for u in fillers[state["done"]:]:
                u()
        # final q-chunk o-proj: each unit becomes TWO independent psum
        # accumulation groups (ec0-1 / ec2-3, merged by the fin ADD) so
        # the scheduler can run the first group's matmuls under the
        # deferred last normalize (a single spanning group would be kept
        # atomic and pushed wholly after it)
        attnp3 = attnps[NQC - 1]
        fst = {}

        def f_mm(t8, co, g):
            ps = fl_pool.tile([P, SC], F32, tag="fl", name="fsc")
            fst[(t8, co, g)] = ps
            for ec in (2 * g, 2 * g + 1):
                nc.tensor.matmul(
                    ps[:], attnp3[ec][:, t8 * P:(t8 + 1) * P],
                    woT_sb[:, ec, co * SC:(co + 1) * SC],
                    start=(ec == 2 * g), stop=(ec == 2 * g + 1),
                )
            if g == 0:
                sbo = ost_pool.tile([P, SC], F32, tag="ost")
                fst[(t8, co, "sb")] = sbo
                nc.vector.tensor_copy(sbo[:], ps[:])

        def f_fin(t8, co):
            psB = fst.pop((t8, co, 1))
            sbo = fst.pop((t8, co, "sb"))
            trow = (NQC - 1) * SC + t8 * P
            nc.vector.tensor_add(sbo[:], sbo[:], psB[:])
            nc.scalar.dma_start(
                outp.ap()[trow:trow + P, co * SC:(co + 1) * SC], sbo[:]
            )

        units = [(t8, co) for t8 in range(SC // P) for co in range(2)]
        f_mm(*units[0], 0)
        f_mm(*units[1], 0)
        while extras:
            extras.pop(0)()
        for i, (t8, co) in enumerate(units):
            f_mm(t8, co, 1)
            f_fin(t8, co)
            if i + 2 < len(units):
                f_mm(*units[i + 2], 0)

    nc.compile()
    return nc


def _prep_inputs(hidden_states, qkv_w, qkv_b, o_w, o_b):
    import ml_dtypes
    bf = ml_dtypes.bfloat16
    hidden_states = np.asarray(hidden_states, dtype=np.float32)
    qkv_w = np.asarray(qkv_w, dtype=np.float32)
    qkv_b = np.asarray(qkv_b, dtype=np.float32)
    o_w = np.asarray(o_w, dtype=np.float32)

    j = np.arange(P)[:, None]
    q = np.arange(P)[None, :]
    msk = (j <= q).astype(bf)

    in_maps = []
    for c in range(NCORES):
        b, g = c // 2, c % 2
        hsT = np.ascontiguousarray(hidden_states[b].T).astype(bf)
        qsel = qkv_w[E * g:E * g + E]
        ksel = qkv_w[C + E * g:C + E * g + E]
        vsel = qkv_w[2 * C + E * g:2 * C + E * g + E]
        wqkvT = np.ascontiguousarray(
            np.concatenate([qsel, ksel, vsel], 0).T
        ).astype(bf)
        woT = np.ascontiguousarray(o_w[:, E * g:E * g + E].T).astype(bf)
        bq = qkv_b[E * g:E * g + E].reshape(4, P).T
        bk = qkv_b[C + E * g:C + E * g + E].reshape(4, P).T
        bv = qkv_b[2 * C + E * g:2 * C + E * g + E]
        bqkv = np.ascontiguousarray(np.concatenate([bq, bk], 1))
        vbias = np.ascontiguousarray(np.tile(bv[None, :], (P, 1)))
        in_maps.append({
            "hsT": hsT, "wqkvT": wqkvT, "woT": woT,
            "bqkv": bqkv, "vbias": vbias, "masks": msk,
        })
    return in_maps


def _get_nc():
    if "nc" not in _cache:
        _cache["nc"] = _build()
    return _cache["nc"]


def _run(in_maps, **kwargs):
    return run_bass_kernel_spmd(
        _get_nc(), in_maps, core_ids=list(range(NCORES)), **kwargs
    )


def kernel(hidden_states, qkv_w, qkv_b, o_w, o_b, **_):
    in_maps = _prep_inputs(hidden_states, qkv_w, qkv_b, o_w, o_b)
    res = _run(in_maps)
    o_b = np.asarray(o_b, dtype=np.float32)
    out = np.empty((B, T, C), dtype=np.float32)
    for b in range(B):
        out[b] = res.results[2 * b]["outp"] + res.results[2 * b + 1]["outp"] + o_b
    return out
